# revision 48
# baseline (speedup 1.0000x reference)
"""Trainium2 Bass kernel for nn_BLCD_Loss (retrieval_knn).

Math: for l2-normalized rows, ||a-b||^2 = 2 - 2*a.b, so all pairwise
distances come from Gram matmuls (bf16 in, fp32 PSUM). Per-core inputs
are column-rolled so the self-pair lands on the local diagonal;
self-exclusion is a -BIG*I accumulated via an extra matmul with
affine_select-generated operands. The anchor (R) and positive (Rt) Grams
share one [64,256] PSUM tile, so column-normalize (DVE mult straight
from PSUM), the sqrt-to-distance activation, and the distance subtract
each run ONCE for both. Column norms come from a ones-matmul replicated
over 64 partitions; per-row scales are built on Pool with
normalize_recip (no DVE reciprocal on the critical path). Top-16
selection is max8 + match_replace + max8 on DVE; dis - dis_t is a PE
selector matmul ([+I|-I]), squared by an otherwise-idle Act Square from
PSUM, and the masked e1 sum is one fused scalar_tensor_tensor.

Output rides a SWDGE kv_writeback whose descriptors are PREPARED during
the input-DMA dead time; after the last compute op a bare trigger_dma
rings the doorbell, skipping the HWDGE issue (625ns) and DGE delay
(650ns) of a plain dma_start. Two post-schedule fixups keep this exact
on hardware and deadlock-free in the cost-model sim: SWDGE ring-slot
waits (satisfied on HW by the un-modeled InstIncSwdgeSem pre-bump) are
neutralized, and the trigger gains explicit engine-lane waits on the
two output writers.

Sharding: 256 anchor rows -> 32 rows on each of 8 cores; each core
returns per-row partials (e1, e2); the host sums them.
"""

import numpy as np

N, D, K = 256, 256, 16
M_MARGIN, T_THRESH, EPS = 0.6, 0.0025, 1e-12
NCORES, RPC = 8, 32
BIG = 1.0e5

_CACHE = {}


def _build():
    import os
    os.environ["TILE_SCHEDULER"] = "asap"
    from concourse import bacc, mybir, tile
    import concourse.bass as bass

    dt = mybir.dt.float32
    bf = mybir.dt.bfloat16
    Alu = mybir.AluOpType
    Act = mybir.ActivationFunctionType

    nc = bacc.Bacc("TRN2", target_bir_lowering=False, debug=False)

    # pA cols (bf16): 0:32 yiLT0 | 32:64 yiLT1 | 64:96 yitT0 | 96:128 yitT1 |
    #                 128:384 yiT rows 0:128 (rolled) | 384:640 rows 128:256
    pA_d = nc.dram_tensor("pA", [128, 640], bf, kind="ExternalInput")
    # kv_writeback layout: [batch=1, d_head_inner=128, d_head_outer=1, n_ctx=2]
    out_d = nc.dram_tensor("out", [1, 128, 1, 2], dt, kind="ExternalOutput")
    kv_sem = nc.alloc_semaphore("kv_dma")

    with tile.TileContext(nc) as tc:
        with (
            tc.tile_pool(name="sb", bufs=1) as sb,
            tc.tile_pool(name="ps", bufs=1, space=bass.MemorySpace.PSUM) as ps,
        ):
            sbA = sb.tile([128, 640], bf)

            # ---- Pool: small constants + on-device identity operands
            cEPS = sb.tile([128, 1], dt)
            nc.gpsimd.memset(cEPS[:], EPS)
            cHALF = sb.tile([128, 1], dt)
            nc.gpsimd.memset(cHALF[:], 0.5)
            zz = sb.tile([RPC, N], bf)
            nc.gpsimd.memset(zz[:], 0.0)
            eyeN = sb.tile([RPC, N], bf)
            nc.gpsimd.affine_select(
                eyeN[:], zz[:], [[1, N]], Alu.not_equal, -BIG,
                base=0, channel_multiplier=-1)
            i64 = sb.tile([RPC, 64], bf)
            nc.gpsimd.affine_select(
                i64[:], zz[:, 0:64], [[1, 64]], Alu.not_equal, 1.0,
                base=0, channel_multiplier=-1)
            # P[p, m] = +1 at p==m, -1 at p==m+32: one matmul computes
            # dis[0:32]-dis[32:64] without partition-offset tensor operands.
            zz64 = sb.tile([64, RPC], bf)
            nc.gpsimd.memset(zz64[:], 0.0)
            pdiff = sb.tile([64, RPC], bf)
            nc.gpsimd.affine_select(
                pdiff[:], zz64[:], [[1, RPC]], Alu.not_equal, 1.0,
                base=0, channel_multiplier=-1)
            pdiff2 = sb.tile([64, RPC], bf)
            nc.gpsimd.affine_select(
                pdiff2[:], pdiff[:], [[1, RPC]], Alu.not_equal, -1.0,
                base=RPC, channel_multiplier=-1)
            ctx_idxs = sb.tile([128, 1], mybir.dt.int32)
            nc.gpsimd.memset(ctx_idxs[:], 0)
            # Staging tile for the output writeback. Only the init memset and
            # the final Pool copy touch it, so the descriptor prep (~1us of
            # Q7 desc-gen) can run NOW, during the input-DMA dead time,
            # without creating write-after-read edges against the compute.
            outstage4 = sb.tile([128, 1, 1, 2], dt)
            outstage = outstage4[:].squeeze(1).squeeze(1)
            nc.gpsimd.memset(outstage, 0.0)
            nc.gpsimd.kv_writeback(
                out_d[:, :, :, :],
                outstage4[:],
                ctx_idxs[:],
                prepare_only=True,
                sem=kv_sem,
            )
            e1dst = outstage4[0:RPC, 0:1, 0:1, 0:1].squeeze(1).squeeze(1)
            e2dst = outstage4[0:RPC, 0:1, 0:1, 1:2].squeeze(1).squeeze(1)

            # ---- SP queue: the yiT block first, then the local columns
            nc.sync.dma_start(sbA[:, 128:640], pA_d[:, 128:640])
            nc.sync.dma_start(sbA[:, 0:128], pA_d[:, 0:128])

            # locals (bf16): 0:32 yiL d0 | 32:64 yit d0 | 64:96 yiL d1 |
            #                96:128 yit d1  (anchor rows as columns)
            yiT0 = sbA[:, 128:384]
            yiT1 = sbA[:, 384:640]
            loc = sbA[:, 0:128]

            # ---- Act: dummy sqrt first so the act table loads at t~0
            dummy = sb.tile([1, 1], dt)
            nc.scalar.activation(dummy[:], cEPS[0:1, :], Act.Sqrt,
                                 bias=cEPS[0:1, :], scale=1.0)

            # ---- DVE: ones + squared yiT halves (bf16, fast)
            ones = sb.tile([128, 64], bf)
            nc.vector.memset(ones[:], 1.0)
            sqA0 = sb.tile([128, D], bf)
            nc.vector.tensor_tensor(sqA0[:], yiT0, yiT0, op=Alu.mult)
            sqA1 = sb.tile([128, D], bf)
            nc.vector.tensor_tensor(sqA1[:], yiT1, yiT1, op=Alu.mult)
            sqL = sb.tile([128, 128], bf)
            nc.vector.tensor_tensor(sqL[:], loc, loc, op=Alu.mult)
            prodL = sb.tile([128, 64], bf)
            nc.vector.tensor_tensor(prodL[:, 0:32], sbA[:, 0:32],
                                    sbA[:, 32:64], op=Alu.mult)
            nc.vector.tensor_tensor(prodL[:, 32:64], sbA[:, 64:96],
                                    sbA[:, 96:128], op=Alu.mult)

            # ---- PE: colsums replicated on 64 parts; fused R|Rt Gram;
            #      row-norm sums in both [64,1] and [32,2] layouts
            ps_s = ps.tile([64, N], dt)
            nc.tensor.matmul(ps_s[:], ones[:], sqA0[:], start=True, stop=False)
            nc.tensor.matmul(ps_s[:], ones[:], sqA1[:], start=False, stop=True)
            ps_n64 = ps.tile([64, 1], dt)
            nc.tensor.matmul(ps_n64[0:32, :], sqL[:, 0:32], ones[:, 0:1],
                             start=True, stop=False, skip_group_check=True)
            nc.tensor.matmul(ps_n64[0:32, :], sqL[:, 64:96], ones[:, 0:1],
                             start=False, stop=True, skip_group_check=True)
            nc.tensor.matmul(ps_n64[32:64, :], sqL[:, 32:64], ones[:, 0:1],
                             start=True, stop=False, skip_group_check=True)
            nc.tensor.matmul(ps_n64[32:64, :], sqL[:, 96:128], ones[:, 0:1],
                             start=False, stop=True, skip_group_check=True)
            ps_n2 = ps.tile([RPC, 2], dt)
            nc.tensor.matmul(ps_n2[:, 0:1], sqL[:, 0:32], ones[:, 0:1],
                             start=True, stop=False, skip_group_check=True)
            nc.tensor.matmul(ps_n2[:, 0:1], sqL[:, 64:96], ones[:, 0:1],
                             start=False, stop=True, skip_group_check=True)
            nc.tensor.matmul(ps_n2[:, 1:2], sqL[:, 32:64], ones[:, 0:1],
                             start=True, stop=False, skip_group_check=True)
            nc.tensor.matmul(ps_n2[:, 1:2], sqL[:, 96:128], ones[:, 0:1],
                             start=False, stop=True, skip_group_check=True)
            ps_dx = ps.tile([RPC, 1], dt)
            nc.tensor.matmul(ps_dx[:], prodL[:, 0:32], ones[:, 0:1],
                             start=True, stop=False)
            nc.tensor.matmul(ps_dx[:], prodL[:, 32:64], ones[:, 0:1],
                             start=False, stop=True)
            ps_R = ps.tile([64, N], dt)
            nc.tensor.matmul(ps_R[:], loc[:, 0:64], yiT0, start=True,
                             stop=False)
            nc.tensor.matmul(ps_R[:], loc[:, 64:128], yiT1, start=False,
                             stop=False)
            nc.tensor.matmul(ps_R[:], i64[:], eyeN[:], start=False, stop=True)

            # ---- norms: t = sqrt(sum + eps), reciprocals; e2 scales via
            #      Pool divides so the DVE queue stays clear for work/top-k
            t_b = sb.tile([64, N], dt)
            nc.scalar.activation(t_b[:], ps_s[:], Act.Sqrt,
                                 bias=cEPS[0:64, :], scale=1.0)
            t64 = sb.tile([64, 1], dt)
            nc.scalar.activation(t64[:], ps_n64[:], Act.Sqrt,
                                 bias=cEPS[0:64, :], scale=1.0)
            inv_b = sb.tile([64, N], dt)
            nc.vector.reciprocal(inv_b[:], t_b[:])
            cNH = sb.tile([64, 1], dt)
            nc.gpsimd.memset(cNH[:], -0.5)
            sc64 = sb.tile([64, 1], dt)
            nc.gpsimd.normalize_recip(sc64[:], cNH[:], t64[:])
            t2 = sb.tile([RPC, 2], dt)
            nc.scalar.activation(t2[:], ps_n2[:], Act.Sqrt,
                                 bias=cEPS[0:RPC, :], scale=1.0)
            tq = sb.tile([RPC, 1], dt)
            nc.gpsimd.tensor_tensor(tq[:], t2[:, 0:1], t2[:, 1:2],
                                    op=Alu.mult)
            scv = sb.tile([RPC, 1], dt)
            nc.gpsimd.normalize_recip(scv[:], cNH[0:RPC, :], tq[:])

            # ---- work = col-normalized [R|Rt] straight from PSUM
            work = sb.tile([64, N], dt)
            nc.vector.tensor_tensor(work[:], ps_R[:], inv_b[:], op=Alu.mult)

            # ---- top-16 threshold per row (self sits at -BIG on the diag)
            m1 = sb.tile([RPC, 8], dt)
            nc.vector.max(out=m1[:], in_=work[0:RPC, :])
            w2 = sb.tile([RPC, N], dt)
            nc.vector.match_replace(
                out=w2[:], in_to_replace=m1[:], in_values=work[0:RPC, :],
                imm_value=-BIG)
            m2 = sb.tile([RPC, 8], dt)
            nc.vector.max(out=m2[:], in_=w2[:])

            # ---- distances for anchors AND positives in one op
            dis = sb.tile([64, N], bf)
            nc.scalar.activation(dis[:], work[:], Act.Sqrt,
                                 bias=cHALF[0:64, :], scale=sc64[:, 0:1])
            dis2 = sb.tile([RPC, 1], dt)
            nc.scalar.activation(dis2[:], m1[:, 0:1], Act.Sqrt,
                                 bias=cHALF[0:RPC, :], scale=sc64[0:RPC, 0:1])

            # ---- e2 = relu(dis_ii + (margin - dis2)) on the slack path
            dis_ii = sb.tile([RPC, 1], dt)
            nc.scalar.activation(dis_ii[:], ps_dx[:], Act.Sqrt,
                                 bias=cHALF[0:RPC, :], scale=scv[:, 0:1])
            bias2 = sb.tile([RPC, 1], dt)
            nc.gpsimd.tensor_scalar(
                bias2[:], dis2[:], -1.0, M_MARGIN, op0=Alu.mult, op1=Alu.add)
            relu_i = nc.scalar.activation(e2dst, dis_ii[:], Act.Relu,
                                          bias=bias2[:], scale=1.0)

            # ---- e1 = sum over neighbors of (dis - dis_t)^2; the subtract
            #      runs on the PE (selector matmul) in parallel with the
            #      DVE top-k chain
            ps_diff = ps.tile([RPC, N], dt)
            nc.tensor.matmul(ps_diff[:], pdiff2[:], dis[:], start=True,
                             stop=True)
            diffsq = sb.tile([RPC, N], bf)
            nc.scalar.activation(diffsq[:], ps_diff[:], Act.Square,
                                 bias=0.0, scale=1.0)
            scr1 = sb.tile([RPC, N], bf)
            scr1_i = nc.vector.scalar_tensor_tensor(
                scr1[:], work[0:RPC, :], m2[:, 7:8], diffsq[:],
                op0=Alu.is_ge, op1=Alu.mult, accum_out=e1dst)

            # scr1/relu write the staging tile directly; the trigger gets
            # explicit lane waits on both writers via the post-pass below.
            trig_i = nc.gpsimd.trigger_dma(count=None)
            trig_name = trig_i.ins.name
            writer_names = [scr1_i.ins.name, relu_i.ins.name]

    # DMASW0 waits are SWDGE ring-slot accounting: on hardware the
    # InstIncSwdgeSem pre-bump (internal ISA fields, +16 at prep position)
    # satisfies them the moment the Pool queue passes it, so they are
    # vacuously true at arrival. The cost-model sim does not model that
    # internal bump, so neutralize the waits (>= 0); actual SWDGE DMA drain
    # is enforced by the Pool ISA ring-drain in the tile epilogue.
    for blk in nc.m.functions[0].blocks:
        for inst in blk.instructions:
            si = inst.sync_info
            if si is None:
                continue
            for w in si.on_wait:
                if w.ant_name is not None and w.ant_name.startswith("DMASW"):
                    w.wait_value = 0

    # Gate the doorbell on the actual output writers: find each writer's
    # engine-lane semaphore and cumulative tick, then append those waits to
    # the trigger (its own Pool-lane wait only covers the prep's desc-gen).
    import copy as _copy
    lane = {}
    for nm in writer_names:
        sem_id = None
        for blk in nc.m.functions[0].blocks:
            for inst in blk.instructions:
                if inst.name != nm or inst.sync_info is None:
                    continue
                for u in inst.sync_info.on_update:
                    if u.update_mode == "sem-inc":
                        sem_id = u.id
        assert sem_id is not None, nm
        cum = 0
        tick = None
        for blk in nc.m.functions[0].blocks:
            for inst in blk.instructions:
                si = inst.sync_info
                if si is None:
                    continue
                for u in si.on_update:
                    if u.id == sem_id and u.update_mode == "sem-inc":
                        cum += u.update_value
                if inst.name == nm:
                    tick = cum
        assert tick is not None, nm
        lane[sem_id] = max(lane.get(sem_id, 0), tick)
    for blk in nc.m.functions[0].blocks:
        for inst in blk.instructions:
            if inst.name != trig_name:
                continue
            si = inst.sync_info
            proto = si.on_wait[0]
            for sem_id, tick in lane.items():
                w = _copy.copy(proto)
                w.id = sem_id
                w.wait_value = tick
                si.on_wait.append(w)

    nc.compile()
    return nc


def _in_maps(yi, yi_t):
    import ml_dtypes
    bf16 = ml_dtypes.bfloat16
    yi = np.ascontiguousarray(np.asarray(yi, np.float32))
    yi_t = np.ascontiguousarray(np.asarray(yi_t, np.float32))
    yiT = yi.T
    maps = []
    for c in range(NCORES):
        r0 = c * RPC
        yiTp = np.roll(yiT, -r0, axis=1)
        pA = np.empty((128, 640), np.float32)
        pA[:, 0:32] = yi[r0:r0 + RPC, 0:128].T
        pA[:, 32:64] = yi_t[r0:r0 + RPC, 0:128].T
        pA[:, 64:96] = yi[r0:r0 + RPC, 128:256].T
        pA[:, 96:128] = yi_t[r0:r0 + RPC, 128:256].T
        pA[:, 128:384] = yiTp[0:128, :]
        pA[:, 384:640] = yiTp[128:256, :]
        maps.append({"pA": pA.astype(bf16)})
    return maps


def kernel(yi, yi_t):
    from concourse.bass_utils import run_bass_kernel_spmd

    if "nc" not in _CACHE:
        _CACHE["nc"] = _build()
    nc = _CACHE["nc"]
    res = run_bass_kernel_spmd(nc, _in_maps(yi, yi_t), list(range(NCORES)))
    total = np.float64(0.0)
    for c in range(NCORES):
        o = np.asarray(res.results[c]["out"]).reshape(128, 2)
        total += np.sum(o[0:RPC, :], dtype=np.float64)
    total -= np.float64(N * K * T_THRESH)
    return np.float32(total)

